# revision 11
# baseline (speedup 1.0000x reference)
"""MoE top-2 routing kernel for Trainium2, expert-parallel over 8 NeuronCores.

Problem (fp32):
  x [2, 2048, 512], gate Wg [512, 8] (+bg), experts W1 [8, 512, 2048] (+b1),
  W2 [8, 2048, 512] (+b2), top-2 softmax-renormalized combine.

Sharding: expert parallelism — core e holds expert e's weights. The gate is
replicated (every core computes the full gate for all 4096 tokens, then keeps
only its own expert's combine column). Each core computes y_e = combine[:, e]
* FFN_e(x) for the tokens it owns and the host unshard step sums the 8
partial outputs.

Matmul precision: the gate matmul runs in full fp32 (so top-2 selection
matches the fp32 reference; a routing flip would be a large error), the FFN
matmuls run as float32r (full PE rate at free-dim >= 256).
"""

import os
import sys

import numpy as np

for _p in ("/opt/trn_rl_repo",):
    if _p not in sys.path and os.path.isdir(_p):
        sys.path.insert(0, _p)

P = 128          # SBUF partitions
D = 512          # d_model
F = 2048         # d_ff
T = 4096         # tokens (B*S)
E = 8            # experts
KD = D // P      # 4  k-tiles over d_model
KF = F // P      # 16 f-tiles over d_ff
NT = T // P      # 32 token tiles
CH = 512         # token chunk width for hT matmuls (>=256 keeps f32r at rate)
NCH = T // CH    # 8 chunks
NCORES = 8

_CACHE = {}


def _build_dense():
    """One SPMD program: dense per-expert FFN over all tokens."""
    from contextlib import ExitStack

    import concourse.bass as bass
    import concourse.mybir as mybir
    import concourse.tile as tile
    from concourse import bacc

    f32 = mybir.dt.float32
    f32r = mybir.dt.float32r
    ts = bass.ts

    nc = bacc.Bacc("TRN2", target_bir_lowering=False)
    xT = nc.declare_dram_parameter("xT", [D, T], f32, isOutput=False)
    wg = nc.declare_dram_parameter("Wg", [D, E], f32, isOutput=False)
    bgbc = nc.declare_dram_parameter("bgbc", [P, E], f32, isOutput=False)
    w1 = nc.declare_dram_parameter("W1e", [D, F], f32r, isOutput=False)
    b1p = nc.declare_dram_parameter("b1p", [P, KF], f32, isOutput=False)
    w2 = nc.declare_dram_parameter("W2e", [F, D], f32r, isOutput=False)
    b2bc = nc.declare_dram_parameter("b2bc", [P, D], f32, isOutput=False)
    esel = nc.declare_dram_parameter("esel", [P, E], f32, isOutput=False)
    y = nc.declare_dram_parameter("y", [T, D], f32, isOutput=True)

    with tile.TileContext(nc) as tc, ExitStack() as ctx:
        consts = ctx.enter_context(tc.tile_pool(name="consts", bufs=1))
        gate_ps = ctx.enter_context(tc.tile_pool(name="gate_ps", bufs=2, space="PSUM"))
        gate_sb = ctx.enter_context(tc.tile_pool(name="gate_sb", bufs=4))
        h_ps = ctx.enter_context(tc.tile_pool(name="h_ps", bufs=3, space="PSUM"))
        h_pool = ctx.enter_context(tc.tile_pool(name="h_pool", bufs=18))
        y_ps = ctx.enter_context(tc.tile_pool(name="y_ps", bufs=2, space="PSUM"))
        y_sb = ctx.enter_context(tc.tile_pool(name="y_sb", bufs=3))
        xtr_pool = ctx.enter_context(tc.tile_pool(name="xtr_pool", bufs=2))

        # ---- resident loads ----
        wg_t = [consts.tile([P, E], f32, name=f"wg{k}", tag=f"wg{k}") for k in range(KD)]
        for k in range(KD):
            nc.sync.dma_start(wg_t[k][:], wg[ts(k, P), :])
        bgbc_t = consts.tile([P, E], f32, tag="bgbc")
        nc.sync.dma_start(bgbc_t[:], bgbc[:, :])
        esel_t = consts.tile([P, E], f32, tag="esel")
        nc.sync.dma_start(esel_t[:], esel[:, :])

        xt_t = [consts.tile([P, T], f32, name=f"xt{k}", tag=f"xt{k}") for k in range(KD)]
        # chunked loads so the gate can start before the whole tile lands
        for k in range(KD):
            for h in range(4):
                nc.sync.dma_start(xt_t[k][:, ts(h, T // 4)], xT[ts(k, P), ts(h, T // 4)])

        w1_t = [consts.tile([P, F], f32r, name=f"w1{k}", tag=f"w1{k}") for k in range(KD)]
        for k in range(KD):
            nc.sync.dma_start(w1_t[k][:], w1[ts(k, P), :])
        w2_t = [consts.tile([P, D], f32r, name=f"w2{f}", tag=f"w2{f}") for f in range(KF)]
        for f in range(KF):
            nc.sync.dma_start(w2_t[f][:], w2[ts(f, P), :])
        b1p_t = consts.tile([P, KF], f32, tag="b1p")
        nc.sync.dma_start(b1p_t[:], b1p[:, :])
        b2bc_t = consts.tile([P, D], f32, tag="b2bc")
        nc.sync.dma_start(b2bc_t[:], b2bc[:, :])

        cvec = consts.tile([P, NT], f32, tag="cvec")  # combine[:, e] per token

        # ---- gate: logits -> exp -> top2 renormalized combine column ----
        for tt in range(NT):
            gps = gate_ps.tile([P, E], f32)
            for k in range(KD):
                nc.tensor.matmul(
                    gps[:],
                    lhsT=xt_t[k][:, ts(tt, P)],
                    rhs=wg_t[k][:],
                    start=(k == 0),
                    stop=(k == KD - 1),
                )
            glog = gate_sb.tile([P, E], f32, tag="glog")
            nc.vector.tensor_tensor(
                out=glog[:], in0=gps[:], in1=bgbc_t[:], op=mybir.AluOpType.add
            )
            probs = gate_sb.tile([P, E], f32, tag="probs")
            nc.scalar.activation(probs[:], glog[:], mybir.ActivationFunctionType.Exp)
            m8 = gate_sb.tile([P, 8], f32, tag="m8")
            nc.vector.max(out=m8[:], in_=probs[:])
            den = gate_sb.tile([P, 1], f32, tag="den")
            nc.vector.tensor_tensor(
                out=den[:], in0=m8[:, 0:1], in1=m8[:, 1:2], op=mybir.AluOpType.add
            )
            rcp = gate_sb.tile([P, 1], f32, tag="rcp")
            nc.vector.reciprocal(rcp[:], den[:])
            # mask of top-2 positions (probs >= second max)
            mask = gate_sb.tile([P, E], f32, tag="mask")
            nc.vector.tensor_scalar(
                out=mask[:],
                in0=probs[:],
                scalar1=m8[:, 1:2],
                scalar2=None,
                op0=mybir.AluOpType.is_ge,
            )
            pm = gate_sb.tile([P, E], f32, tag="pm")
            nc.vector.tensor_tensor(
                out=pm[:], in0=probs[:], in1=mask[:], op=mybir.AluOpType.mult
            )
            pe_ = gate_sb.tile([P, E], f32, tag="pe_")
            nc.vector.tensor_tensor(
                out=pe_[:], in0=pm[:], in1=esel_t[:], op=mybir.AluOpType.mult
            )
            csum = gate_sb.tile([P, 1], f32, tag="csum")
            nc.vector.tensor_reduce(
                out=csum[:],
                in_=pe_[:],
                axis=mybir.AxisListType.X,
                op=mybir.AluOpType.add,
            )
            nc.vector.tensor_scalar_mul(cvec[:, tt : tt + 1], csum[:], rcp[:])

        # ---- FFN: hT = relu(W1.T x + b1); y = hT.T W2 + b2; y *= combine ----
        for cc in range(NCH):
            xtr = []
            for k in range(KD):
                xr = xtr_pool.tile([P, CH], f32r, name=f"xr{k}", tag=f"xr{k}")
                nc.vector.tensor_copy(xr[:], xt_t[k][:, ts(cc, CH)])
                xtr.append(xr)
            hts = []
            for ft in range(KF):
                hp = h_ps.tile([P, CH], f32)
                for k in range(KD):
                    nc.tensor.matmul(
                        hp[:],
                        lhsT=w1_t[k][:, ts(ft, P)],
                        rhs=xtr[k][:],
                        start=(k == 0),
                        stop=(k == KD - 1),
                    )
                hs = h_pool.tile([P, CH], f32r, tag="hs")
                nc.scalar.activation(
                    hs[:],
                    hp[:],
                    mybir.ActivationFunctionType.Relu,
                    bias=b1p_t[:, ft : ft + 1],
                )
                hts.append(hs)
            for st in range(CH // P):
                tt = cc * (CH // P) + st
                yp = y_ps.tile([P, D], f32)
                for fk in range(KF):
                    nc.tensor.matmul(
                        yp[:],
                        lhsT=hts[fk][:, ts(st, P)],
                        rhs=w2_t[fk][:],
                        start=(fk == 0),
                        stop=(fk == KF - 1),
                    )
                ys = y_sb.tile([P, D], f32, tag="ys")
                nc.vector.tensor_tensor(
                    out=ys[:], in0=yp[:], in1=b2bc_t[:], op=mybir.AluOpType.add
                )
                nc.vector.tensor_scalar_mul(ys[:], ys[:], cvec[:, tt : tt + 1])
                nc.sync.dma_start(y[ts(tt, P), :], ys[:])

    nc.compile()
    return nc


CPAD = 1280          # per-expert token capacity (mean load 1024, ~8.5 sigma)
BIG = 1 << 20        # scatter position for unrouted tokens (beyond bounds check)
GT = CPAD // P       # 10 gather tiles
CHUNKS = [(0, 512), (512, 512), (1024, 256)]  # (start, size) token chunks


def _build_sparse():
    """Sparse expert-parallel MoE: on-device top-2 routing + token gather.

    Per core: full fp32 gate over all 4096 tokens (batched top-2/combine on
    DVE), position build via triangular-matrix matmuls (exclusive prefix
    counts), indirect-DMA scatter of (token_id, combine) pairs into a compact
    DRAM list, indirect-DMA row gather of the routed tokens, PE-transpose,
    fp32r FFN over CPAD tokens, and indirect-DMA scatter of the weighted
    expert output back to token rows.
    """
    from contextlib import ExitStack

    import concourse.bass as bass
    import concourse.mybir as mybir
    import concourse.tile as tile
    from concourse import bacc

    f32 = mybir.dt.float32
    f32r = mybir.dt.float32r
    i32 = mybir.dt.int32
    u32 = mybir.dt.uint32
    ts = bass.ts
    Alu = mybir.AluOpType
    Act = mybir.ActivationFunctionType

    nc = bacc.Bacc("TRN2", target_bir_lowering=False)
    xT = nc.declare_dram_parameter("xT", [D, T], f32, isOutput=False)
    xr = nc.declare_dram_parameter("xr", [T + 1, D], f32, isOutput=False)
    wg = nc.declare_dram_parameter("Wg", [D, E], f32, isOutput=False)
    bgbc32 = nc.declare_dram_parameter("bgbc32", [P, NT * E], f32, isOutput=False)
    esel256 = nc.declare_dram_parameter("esel256", [P, NT * E], f32, isOutput=False)
    w1 = nc.declare_dram_parameter("W1e", [D, F], f32r, isOutput=False)
    b1p = nc.declare_dram_parameter("b1p", [P, KF], f32, isOutput=False)
    w2 = nc.declare_dram_parameter("W2e", [F, D], f32r, isOutput=False)
    b2bc = nc.declare_dram_parameter("b2bc", [P, D], f32, isOutput=False)
    lstrict = nc.declare_dram_parameter("Lstrict", [P, P], f32, isOutput=False)
    onesm = nc.declare_dram_parameter("ones", [P, P], f32, isOutput=False)
    ident = nc.declare_dram_parameter("ident", [P, P], f32, isOutput=False)
    tvals = nc.declare_dram_parameter("tvals", [P, NT], i32, isOutput=False)
    prefill = nc.declare_dram_parameter(
        "prefill", [P, CPAD * 2 // P], u32, isOutput=False
    )
    y = nc.declare_dram_parameter("y", [T + 1, D], f32, isOutput=True)

    idxc = nc.dram_tensor("idxc", [CPAD + 1, 2], u32)

    with tile.TileContext(nc) as tc, ExitStack() as ctx:
        consts = ctx.enter_context(tc.tile_pool(name="consts", bufs=1))
        small_ps = ctx.enter_context(tc.tile_pool(name="small_ps", bufs=2, space="PSUM"))
        gate_sb = ctx.enter_context(tc.tile_pool(name="gate_sb", bufs=2))
        tr_ps = ctx.enter_context(tc.tile_pool(name="tr_ps", bufs=2, space="PSUM"))
        h_ps = ctx.enter_context(tc.tile_pool(name="h_ps", bufs=2, space="PSUM"))
        y_ps = ctx.enter_context(tc.tile_pool(name="y_ps", bufs=2, space="PSUM"))
        xt_pool_cm = tc.tile_pool(name="xt_pool", bufs=1)
        xt_pool = xt_pool_cm.__enter__()

        # ---- resident loads (order matters: gate deps first) ----
        wg_t = [consts.tile([P, E], f32, name=f"wg{k}", tag=f"wg{k}") for k in range(KD)]
        for k in range(KD):
            nc.sync.dma_start(wg_t[k][:], wg[ts(k, P), :])
        xt_t = [xt_pool.tile([P, T], f32, name=f"xt{k}", tag=f"xt{k}") for k in range(KD)]
        for k in range(KD):
            for h in range(4):
                nc.sync.dma_start(xt_t[k][:, ts(h, T // 4)], xT[ts(k, P), ts(h, T // 4)])
        bgbc_t = consts.tile([P, NT * E], f32, tag="bgbc")
        nc.sync.dma_start(bgbc_t[:], bgbc32[:, :])
        esel_t = consts.tile([P, NT * E], f32, tag="esel")
        nc.sync.dma_start(esel_t[:], esel256[:, :])
        lstrict_t = consts.tile([P, P], f32, tag="lstrict")
        nc.sync.dma_start(lstrict_t[:], lstrict[:, :])
        ones_t = consts.tile([P, P], f32, tag="ones")
        nc.sync.dma_start(ones_t[:], onesm[:, :])
        ident_t = consts.tile([P, P], f32, tag="ident")
        nc.sync.dma_start(ident_t[:], ident[:, :])
        tvals_t = consts.tile([P, NT], i32, tag="tvals")
        nc.sync.dma_start(tvals_t[:], tvals[:, :])
        prefill_t = consts.tile([P, CPAD * 2 // P], u32, tag="prefill")
        nc.sync.dma_start(prefill_t[:], prefill[:, :])

        w1_t = [consts.tile([P, F], f32r, name=f"w1{k}", tag=f"w1{k}") for k in range(KD)]
        for k in range(KD):
            nc.sync.dma_start(w1_t[k][:], w1[ts(k, P), :])
        w2_t = [consts.tile([P, D], f32r, name=f"w2{f}", tag=f"w2{f}") for f in range(KF)]
        for f in range(KF):
            nc.sync.dma_start(w2_t[f][:], w2[ts(f, P), :])
        b1p_t = consts.tile([P, KF], f32, tag="b1p")
        nc.sync.dma_start(b1p_t[:], b1p[:, :])
        b2bc_t = consts.tile([P, D], f32, tag="b2bc")
        nc.sync.dma_start(b2bc_t[:], b2bc[:, :])

        # prefill the compact index list with (dump_token=T, combine=0)
        nc.sync.dma_start(
            idxc[:CPAD, :].rearrange("(p a) b -> p (a b)", p=P), prefill_t[:]
        )
        nc.sync.dma_start(idxc[CPAD : CPAD + 1, :], prefill_t[0:1, 0:2])

        # ---- gate: fp32 logits for all tokens, batched top-2 combine ----
        logits = consts.tile([P, NT * E], f32, tag="logits")
        for g8 in range(NT // 4):
            gps = small_ps.tile([P, 4 * E], f32, tag="sps", name="gps")
            for j in range(4):
                tt = g8 * 4 + j
                for k in range(KD):
                    nc.tensor.matmul(
                        gps[:, ts(j, E)],
                        lhsT=xt_t[k][:, ts(tt, P)],
                        rhs=wg_t[k][:],
                        start=(k == 0),
                        stop=(k == KD - 1),
                    )
            nc.vector.tensor_tensor(
                out=logits[:, ts(g8, 4 * E)],
                in0=gps[:],
                in1=bgbc_t[:, ts(g8, 4 * E)],
                op=Alu.add,
            )

        xt_pool_cm.__exit__(None, None, None)
        h_pool = ctx.enter_context(tc.tile_pool(name="h_pool", bufs=17))
        y_sb = ctx.enter_context(tc.tile_pool(name="y_sb", bufs=3))
        xg_pool = ctx.enter_context(tc.tile_pool(name="xg_pool", bufs=5))
        xgt_pool = ctx.enter_context(tc.tile_pool(name="xgt_pool", bufs=6))

        probs = consts.tile([P, NT * E], f32, tag="probs")
        nc.scalar.activation(probs[:], logits[:], Act.Exp)
        p3 = probs[:].rearrange("p (s e) -> p s e", e=E)

        m1 = gate_sb.tile([P, NT], f32, tag="m1")
        nc.vector.tensor_reduce(out=m1[:], in_=p3, axis=mybir.AxisListType.X, op=Alu.max)
        # knock out the max, then find the second max
        noteq = gate_sb.tile([P, NT * E], f32, tag="noteq")
        nc.vector.tensor_tensor(
            out=noteq[:].rearrange("p (s e) -> p s e", e=E),
            in0=p3,
            in1=m1[:].broadcast_to([P, NT, E]),
            op=Alu.is_lt,
        )
        pm2 = gate_sb.tile([P, NT * E], f32, tag="pm2")
        nc.vector.tensor_tensor(
            out=pm2[:], in0=probs[:], in1=noteq[:], op=Alu.mult
        )
        m2 = gate_sb.tile([P, NT], f32, tag="m2")
        nc.vector.tensor_reduce(
            out=m2[:], in_=pm2[:].rearrange("p (s e) -> p s e", e=E),
            axis=mybir.AxisListType.X, op=Alu.max,
        )
        den = gate_sb.tile([P, NT], f32, tag="den")
        nc.vector.tensor_tensor(out=den[:], in0=m1[:], in1=m2[:], op=Alu.add)
        rcp = gate_sb.tile([P, NT], f32, tag="rcp")
        nc.vector.reciprocal(rcp[:], den[:])
        # top-2 mask: probs >= m2
        mask12 = gate_sb.tile([P, NT * E], f32, tag="mask12")
        nc.vector.tensor_tensor(
            out=mask12[:].rearrange("p (s e) -> p s e", e=E),
            in0=p3,
            in1=m2[:].broadcast_to([P, NT, E]),
            op=Alu.is_ge,
        )
        pm = gate_sb.tile([P, NT * E], f32, tag="pm")
        nc.vector.tensor_tensor(out=pm[:], in0=probs[:], in1=mask12[:], op=Alu.mult)
        psel = gate_sb.tile([P, NT * E], f32, tag="psel")
        nc.vector.tensor_tensor(out=psel[:], in0=pm[:], in1=esel_t[:], op=Alu.mult)
        csum = gate_sb.tile([P, NT], f32, tag="csum")
        nc.vector.tensor_reduce(
            out=csum[:], in_=psel[:].rearrange("p (s e) -> p s e", e=E),
            axis=mybir.AxisListType.X, op=Alu.add,
        )
        cvec = consts.tile([P, NT], f32, tag="cvec")
        nc.vector.tensor_tensor(out=cvec[:], in0=csum[:], in1=rcp[:], op=Alu.mult)
        mask_e = consts.tile([P, NT], f32, tag="mask_e")
        nc.vector.tensor_scalar(
            out=mask_e[:], in0=cvec[:], scalar1=0.0, scalar2=None, op0=Alu.is_gt
        )

        # ---- compact position build (exclusive prefix over token order) ----
        colsum_ps = small_ps.tile([NT, 1], f32, tag="sps", name="colsum_ps")
        nc.tensor.matmul(
            colsum_ps[:], lhsT=mask_e[:], rhs=ones_t[:, 0:1], start=True, stop=True
        )
        colsum_sb = gate_sb.tile([NT, 1], f32, tag="colsum_sb")
        nc.vector.tensor_copy(colsum_sb[:], colsum_ps[:])
        off_ps = small_ps.tile([1, NT], f32, tag="sps", name="off_ps")
        nc.tensor.matmul(
            off_ps[:], lhsT=colsum_sb[:], rhs=lstrict_t[:NT, :NT], start=True, stop=True
        )
        off_sb = gate_sb.tile([1, NT], f32, tag="off_sb")
        nc.vector.tensor_copy(off_sb[:], off_ps[:])
        pos_ps = small_ps.tile([P, NT], f32, tag="sps", name="pos_ps")
        nc.tensor.matmul(
            pos_ps[:], lhsT=lstrict_t[:], rhs=mask_e[:], start=True, stop=False
        )
        nc.tensor.matmul(
            pos_ps[:], lhsT=ones_t[0:1, :], rhs=off_sb[:], start=False, stop=True
        )
        posm = gate_sb.tile([P, NT], f32, tag="posm")
        nc.vector.scalar_tensor_tensor(
            out=posm[:], in0=pos_ps[:], scalar=float(-CPAD), in1=mask_e[:],
            op0=Alu.add, op1=Alu.mult,
        )
        nc.vector.tensor_scalar_add(posm[:], posm[:], float(CPAD))
        nc.vector.tensor_scalar_min(posm[:], posm[:], float(CPAD))
        pos_i = gate_sb.tile([P, NT], i32, tag="pos_i")
        nc.vector.tensor_copy(pos_i[:], posm[:])

        # ---- scatter (token_id, combine) into the compact list ----
        packed = gate_sb.tile([P, NT * 2], u32, tag="packed")
        pk3 = packed[:].rearrange("p (s two) -> p s two", two=2)
        nc.vector.tensor_copy(pk3[:, :, 0], tvals_t[:])
        nc.vector.tensor_copy(pk3[:, :, 1], cvec[:].bitcast(u32))
        nc.gpsimd.indirect_dma_start(
            out=idxc[:, :],
            out_offset=bass.IndirectOffsetOnAxis(ap=pos_i[:], axis=0),
            in_=pk3,
            in_offset=None,
        )

        # ---- gather routed tokens + their combine weights ----
        idx_sb = []
        for g in range(GT):
            it = xg_pool.tile([P, 2], u32, name=f"idx{g}", tag=f"idx{g}", bufs=1)
            nc.sync.dma_start(it[:], idxc[ts(g, P), :])
            idx_sb.append(it)
        xg_sb = []
        for g in range(GT):
            xg = xg_pool.tile([P, D], f32, name="xg", tag="xg")
            nc.gpsimd.indirect_dma_start(
                out=xg[:],
                out_offset=None,
                in_=xr[:, :],
                in_offset=bass.IndirectOffsetOnAxis(
                    ap=idx_sb[g][:, 0:1].bitcast(i32), axis=0
                ),
            )
            xg_sb.append(xg)

        # ---- FFN on gathered tokens ----
        for q0, qn in CHUNKS:
            g0 = q0 // P
            ng = qn // P
            # PE transpose xg -> xgT (f32r via the PSUM eviction copy)
            xgt = []
            for k in range(KD):
                tp = tr_ps.tile([P, qn], f32, tag="tp", name="tp")
                for gl in range(ng):
                    nc.tensor.transpose(
                        tp[:, ts(gl, P)],
                        xg_sb[g0 + gl][:, ts(k, P)],
                        ident_t[:],
                    )
                xt_r = xgt_pool.tile([P, qn], f32r, tag="xgt", name="xgt")
                nc.vector.tensor_copy(xt_r[:], tp[:])
                xgt.append(xt_r)
            hts = []
            for ft in range(KF):
                hp = h_ps.tile([P, qn], f32)
                for k in range(KD):
                    nc.tensor.matmul(
                        hp[:],
                        lhsT=w1_t[k][:, ts(ft, P)],
                        rhs=xgt[k][:],
                        start=(k == 0),
                        stop=(k == KD - 1),
                    )
                hs = h_pool.tile([P, qn], f32r, tag="hs")
                nc.scalar.activation(
                    hs[:], hp[:], Act.Relu, bias=b1p_t[:, ft : ft + 1]
                )
                hts.append(hs)
            for gl in range(ng):
                g = g0 + gl
                yp = y_ps.tile([P, D], f32)
                for fk in range(KF):
                    nc.tensor.matmul(
                        yp[:],
                        lhsT=hts[fk][:, ts(gl, P)],
                        rhs=w2_t[fk][:],
                        start=(fk == 0),
                        stop=(fk == KF - 1),
                    )
                ys = y_sb.tile([P, D], f32, tag="ys")
                nc.vector.tensor_tensor(
                    out=ys[:], in0=yp[:], in1=b2bc_t[:], op=Alu.add
                )
                nc.vector.tensor_scalar_mul(
                    ys[:], ys[:], idx_sb[g][:, 1:2].bitcast(f32)
                )
                nc.gpsimd.indirect_dma_start(
                    out=y[:, :],
                    out_offset=bass.IndirectOffsetOnAxis(
                        ap=idx_sb[g][:, 0:1].bitcast(i32), axis=0
                    ),
                    in_=ys[:],
                    in_offset=None,
                )

    nc.compile()
    return nc


def _get_program():
    mode = os.environ.get("KERNEL_MODE", "sparse")
    key = f"nc_{mode}"
    if key not in _CACHE:
        _CACHE[key] = _build_sparse() if mode == "sparse" else _build_dense()
    return _CACHE[key]


def _make_in_maps(x, Wg, bg, W1, b1, W2, b2):
    x = np.ascontiguousarray(np.asarray(x, dtype=np.float32).reshape(T, D))
    Wg = np.ascontiguousarray(np.asarray(Wg, dtype=np.float32))
    bg = np.asarray(bg, dtype=np.float32)
    W1 = np.asarray(W1, dtype=np.float32)
    b1 = np.asarray(b1, dtype=np.float32)
    W2 = np.asarray(W2, dtype=np.float32)
    b2 = np.asarray(b2, dtype=np.float32)

    xt = np.ascontiguousarray(x.T)  # [D, T]
    mode = os.environ.get("KERNEL_MODE", "sparse")

    if mode == "dense":
        bgbc = np.ascontiguousarray(np.broadcast_to(bg, (P, E)))
        in_maps = []
        for e in range(NCORES):
            esel = np.zeros((P, E), dtype=np.float32)
            esel[:, e] = 1.0
            in_maps.append(
                {
                    "xT": xt,
                    "Wg": Wg,
                    "bgbc": bgbc,
                    "W1e": np.ascontiguousarray(W1[e]),
                    "b1p": np.ascontiguousarray(b1[e].reshape(KF, P).T),
                    "W2e": np.ascontiguousarray(W2[e]),
                    "b2bc": np.ascontiguousarray(np.broadcast_to(b2[e], (P, D))),
                    "esel": esel,
                }
            )
        return in_maps

    xrow = np.vstack([x, np.zeros((1, D), np.float32)])  # [T+1, D], dump row
    bgbc32 = np.ascontiguousarray(np.tile(bg, (P, NT)))  # [P, NT*E]
    lstrict = np.triu(np.ones((P, P), np.float32), 1)
    ones = np.ones((P, P), np.float32)
    ident = np.eye(P, dtype=np.float32)
    tvals = (np.arange(NT, dtype=np.int32)[None, :] * P) + np.arange(
        P, dtype=np.int32
    )[:, None]  # tvals[p, j] = j*128 + p
    pre = np.zeros((CPAD, 2), np.uint32)
    pre[:, 0] = T  # dump token id
    prefill = np.ascontiguousarray(pre.reshape(P, CPAD * 2 // P))

    in_maps = []
    for e in range(NCORES):
        esel = np.zeros(E, dtype=np.float32)
        esel[e] = 1.0
        in_maps.append(
            {
                "xT": xt,
                "xr": xrow,
                "Wg": Wg,
                "bgbc32": bgbc32,
                "esel256": np.ascontiguousarray(np.tile(esel, (P, NT))),
                "W1e": np.ascontiguousarray(W1[e]),
                "b1p": np.ascontiguousarray(b1[e].reshape(KF, P).T),
                "W2e": np.ascontiguousarray(W2[e]),
                "b2bc": np.ascontiguousarray(np.broadcast_to(b2[e], (P, D))),
                "Lstrict": lstrict,
                "ones": ones,
                "ident": ident,
                "tvals": np.ascontiguousarray(tvals),
                "prefill": prefill,
            }
        )
    return in_maps


def run(inputs, trace=False):
    """Run the kernel; returns (out [2,2048,512] f32, exec_time_ns or None)."""
    from concourse.bass_utils import run_bass_kernel_spmd

    nc = _get_program()
    in_maps = _make_in_maps(**inputs)
    res = run_bass_kernel_spmd(nc, in_maps, list(range(NCORES)), trace=trace)
    acc = np.zeros((T, D), dtype=np.float32)
    for r in res.results:
        yc = np.asarray(r["y"], dtype=np.float32)
        acc += yc.reshape(-1, D)[:T]
    return acc.reshape(2, 2048, D), res.exec_time_ns


def kernel(x, Wg, bg, W1, b1, W2, b2):
    out, _ = run(dict(x=x, Wg=Wg, bg=bg, W1=W1, b1=b1, W2=W2, b2=b2))
    return out
